# revision 1
# baseline (speedup 1.0000x reference)
"""Causal self-attention (B=4, T=2048, C=256, H=8, HD=32) on 8 NeuronCores.

Sharding: core = 2*b + g  (b = batch 0..3, g = head-group 0..1; each group =
4 heads = 128 channels). Each core computes, for its batch b and its 4 heads:
  qkv projection (its slice of w_qkv), causal softmax attention, and a
  partial output projection y_partial = att_out @ w_proj[group rows, :].
Host sums the two partials per batch and adds b_proj.

Per-core kernel layout (everything fp32 in HBM):
  xT   [128(c_in), 2, 2048]   x[b] transposed via PE (fp32r)
  qT   [128(4 heads x 32 d), 2048]  = (x@wq).T   (fp32r)
  kT   [128, 2048]                               (fp32r)
  vaug [128(tk), 16, 256]  V in natural layout + ones col per head (fp16)
         pair P in cols [128P,128P+128): local head l at [64l,64l+32)=V,
         col 64l+32 = ones (softmax denominator via matmul), rest zeros.
  Main loop over 4 query chunks of 512, tk tiles i<=4c+3:
    S^T[tk,tq] psum [128,4,512] <- 4x row-tiled (K=32) fp32r matmuls
    expS fp16 <- ACT exp(scale*psum) (one ACTIVATE per (c,i) over 4 heads)
    causal mask on diag blocks via gpsimd affine_select (post-exp, fill 0)
    O^T accum psum [128,512] x2 <- fp16 col-tiled (M=64) matmuls over i
  Normalize O^T rows by matmul-broadcast reciprocal denominators, then
  y = O^T.T @ wp per 128-row tile (fp32r) -> DMA out.
"""

import math

import numpy as np

B, T, C = 4, 2048, 256
H, HD = 8, 32
NCORES = 8
SCALE = 1.0 / math.sqrt(HD)

_NC_CACHE = {}


def _build(has_bias: bool):
    import concourse.bass as bass  # noqa: F401
    import concourse.mybir as mybir
    import concourse.tile as tile
    from concourse import bacc

    F32 = mybir.dt.float32
    F32R = mybir.dt.float32r
    F16 = mybir.dt.float16
    AF = mybir.ActivationFunctionType
    ALU = mybir.AluOpType

    nc = bacc.Bacc("TRN2", target_bir_lowering=False, debug=False, num_devices=NCORES)

    x_d = nc.declare_dram_parameter("x", [T, C], F32, isOutput=False)
    wqk_d = nc.declare_dram_parameter("wqk", [C, 256], F32, isOutput=False)
    wva_d = nc.declare_dram_parameter("wva", [C, 256], F32, isOutput=False)
    wp_d = nc.declare_dram_parameter("wp", [128, C], F32, isOutput=False)
    vrow_d = nc.declare_dram_parameter("vrow", [1, 256], F32, isOutput=False)
    ed_d = nc.declare_dram_parameter("ed", [128, 128], F32, isOutput=False)
    bq_d = nc.declare_dram_parameter("bq", [128, 1], F32, isOutput=False)
    bk_d = nc.declare_dram_parameter("bk", [128, 1], F32, isOutput=False)
    y_d = nc.declare_dram_parameter("y", [T, C], F32, isOutput=True)

    NT = T // 128  # 16 tk/tq tiles
    NCH = T // 512  # 4 chunks

    with tile.TileContext(nc) as tc:
        with (
            tc.tile_pool(name="cst", bufs=1) as cst,
            tc.tile_pool(name="xin", bufs=2) as xin,
            tc.tile_pool(name="expp", bufs=4) as expp,
            tc.tile_pool(name="yout", bufs=2) as yout,
            tc.tile_pool(name="smt", bufs=2) as smt,
            tc.tile_pool(name="pss", bufs=2, space="PSUM") as pss,
            tc.tile_pool(name="psS", bufs=1, space="PSUM") as psS,
            tc.tile_pool(name="psv", bufs=2, space="PSUM") as psv,
        ):
            # ---------------- setup ----------------
            from concourse.masks import make_identity

            ident = cst.tile([128, 128], F32)
            make_identity(nc, ident[:])

            # E selector [128,128]: E[32h, 32h:32h+32] = 1 (host-prepared)
            E = cst.tile([128, 128], F32)
            nc.sync.dma_start(E[:], ed_d[:])

            # weights: DMA to fp32 staging, round-copy to fp32r
            wqk_st = cst.tile([128, 2, 256], F32)
            wva_st = cst.tile([128, 2, 256], F32)
            wp_st = cst.tile([128, 256], F32)
            nc.sync.dma_start(wqk_st[:], wqk_d[:].rearrange("(a p) m -> p a m", p=128))
            nc.sync.dma_start(wva_st[:], wva_d[:].rearrange("(a p) m -> p a m", p=128))
            nc.sync.dma_start(wp_st[:], wp_d[:])
            wqk_sb = cst.tile([128, 2, 256], F32R)
            wva_sb = cst.tile([128, 2, 256], F16)
            nc.vector.tensor_copy(wqk_sb[:], wqk_st[:])
            nc.vector.tensor_copy(wva_sb[:], wva_st[:])

            # v-bias (+ones) broadcast tile via K=1 outer-product matmul
            vrow_sb = cst.tile([1, 256], F32)
            nc.sync.dma_start(vrow_sb[:], vrow_d[:])
            ones1 = cst.tile([1, 128], F32)
            nc.vector.memset(ones1[:], 1.0)
            pbv = pss.tile([128, 512], F32, tag="sm")
            nc.tensor.matmul(
                pbv[:, :256], ones1[:], vrow_sb[:], start=True, stop=True
            )
            bvb = cst.tile([128, 256], F32)
            nc.vector.tensor_copy(bvb[:], pbv[:, :256])

            bq_sb = cst.tile([128, 1], F32)
            bk_sb = cst.tile([128, 1], F32)
            nc.sync.dma_start(bq_sb[:], bq_d[:])
            nc.sync.dma_start(bk_sb[:], bk_d[:])

            # ---------------- x load + transpose ----------------
            xT = cst.tile([128, 2, T], F32R)
            xTh = cst.tile([128, 2, T], F16)
            xtiles = []
            for a in range(NT // 4):
                xt = xin.tile([128, 4, 256], F32, tag="xin")
                nc.sync.dma_start(
                    xt[:],
                    x_d[512 * a : 512 * a + 512, :].rearrange(
                        "(t p) c -> p t c", p=128
                    ),
                )
                xtiles.append(xt)
            for t in range(NT):
                xt = xtiles[t // 4]
                for ko in range(2):
                    ptr = pss.tile([128, 512], F32, tag="sm")
                    nc.tensor.transpose(
                        ptr[:, :128],
                        xt[:, t % 4, 128 * ko : 128 * ko + 128],
                        ident[:],
                    )
                    nc.vector.tensor_copy(
                        xT[:, ko, 128 * t : 128 * t + 128], ptr[:, :128]
                    )
                    nc.vector.tensor_copy(
                        xTh[:, ko, 128 * t : 128 * t + 128], ptr[:, :128]
                    )

            qT = cst.tile([128, T], F16)
            kT = cst.tile([128, T], F16)
            vaug = cst.tile([128, NT, 256], F16)
            otr = cst.tile([128, T], F32)
            otn = cst.tile([128, T], F32)
            # denominators land on partitions 0/32/64/96; other rows stay 1.0
            dnw = cst.tile([128, 512], F32)
            nc.vector.memset(dnw[:], 1.0)

            def qk_chunk(n):
                for dest, bias, wc in ((qT, bq_sb, 0), (kT, bk_sb, 128)):
                    pq = pss.tile([128, 512], F32, tag="sm")
                    for ko in range(2):
                        nc.tensor.matmul(
                            pq[:],
                            wqk_sb[:, ko, wc : wc + 128],
                            xT[:, ko, 512 * n : 512 * n + 512],
                            start=(ko == 0),
                            stop=(ko == 1),
                        )
                    if has_bias:
                        nc.vector.tensor_scalar(
                            dest[:, 512 * n : 512 * n + 512],
                            pq[:],
                            bias[:, 0:1],
                            None,
                            ALU.add,
                        )
                    else:
                        nc.vector.tensor_copy(
                            dest[:, 512 * n : 512 * n + 512], pq[:]
                        )

            def v_tile(t):
                pv_ = pss.tile([128, 512], F32, tag="sm")
                for ko in range(2):
                    nc.tensor.matmul(
                        pv_[:, :256],
                        xTh[:, ko, 128 * t : 128 * t + 128],
                        wva_sb[:, ko, :],
                        start=(ko == 0),
                        stop=(ko == 1),
                    )
                # add v-bias broadcast and the ones/zeros pattern; fp16 out
                nc.vector.tensor_tensor(
                    vaug[:, t, :], pv_[:, :256], bvb[:], ALU.add
                )

            # ---------------- main attention loop ----------------
            for c in range(NCH):
                qk_chunk(c)
                for t in range(4 * c, 4 * c + 4):
                    v_tile(t)

                pva = psv.tile([128, 512], F32, tag="pv")
                pvb = psv.tile([128, 512], F32, tag="pv")
                pv = (pva, pvb)
                ilast = 4 * c + 3
                for i in range(0, 4 * c + 4):
                    off = max(0, (i - 4 * c) * 128)
                    sp = psS.tile([128, 4, 512], F32, tag="S")
                    for h in range(4):
                        nc.tensor.matmul(
                            sp[:, h, off:512],
                            kT[32 * h : 32 * h + 32, 128 * i : 128 * i + 128],
                            qT[
                                32 * h : 32 * h + 32,
                                512 * c + off : 512 * c + 512,
                            ],
                            start=True,
                            stop=True,
                            tile_position=(32 * h, 0),
                        )
                    ex = expp.tile([128, 4, 512], F16, tag="ex")
                    nc.scalar.activation(
                        ex[:, :, off:512], sp[:, :, off:512], AF.Exp, scale=SCALE
                    )
                    if i >= 4 * c:
                        # causal mask on the diagonal 128x128 block of each head
                        nc.gpsimd.affine_select(
                            out=ex[:, :, off : off + 128],
                            in_=ex[:, :, off : off + 128],
                            compare_op=ALU.is_ge,
                            fill=0.0,
                            base=0,
                            channel_multiplier=-1,
                            pattern=[[0, 4], [1, 128]],
                        )
                    for pi in range(2):
                        for l in range(2):
                            nc.tensor.matmul(
                                pv[pi][64 * l : 64 * l + 64, off:512],
                                vaug[:, i, 128 * pi + 64 * l : 128 * pi + 64 * l + 64],
                                ex[:, 2 * pi + l, off:512],
                                start=(i == 0),
                                stop=(i == ilast),
                                tile_position=(0, 64 * l),
                            )

                # unload O^T (rows) and denominators
                for pi in range(2):
                    for l in range(2):
                        h = 2 * pi + l
                        nc.vector.tensor_copy(
                            otr[32 * h : 32 * h + 32, 512 * c : 512 * c + 512],
                            pv[pi][64 * l : 64 * l + 32, :],
                        )
                    for l in range(2):
                        h = 2 * pi + l
                        nc.vector.tensor_copy(
                            dnw[32 * h : 32 * h + 1, :],
                            pv[pi][64 * l + 32 : 64 * l + 33, :],
                        )

                # normalize: otn = otr * broadcast(1/dn)
                drc = smt.tile([128, 512], F32, tag="drc")
                scr = smt.tile([128, 512], F32, tag="scr")
                nc.vector.reciprocal_approx_accurate(drc[:], dnw[:], scr[:])
                pb = pss.tile([128, 512], F32, tag="sm")
                nc.tensor.matmul(pb[:], E[:], drc[:], start=True, stop=True)
                nc.vector.tensor_tensor(
                    otn[:, 512 * c : 512 * c + 512],
                    otr[:, 512 * c : 512 * c + 512],
                    pb[:],
                    ALU.mult,
                )

                # projection for this chunk's 4 query tiles
                yt = yout.tile([128, 4, 256], F32, tag="yt")
                for mi in range(4):
                    m = 4 * c + mi
                    py_ = pss.tile([128, 512], F32, tag="sm")
                    nc.tensor.matmul(
                        py_[:, :256],
                        otn[:, 128 * m : 128 * m + 128],
                        wp_st[:],
                        start=True,
                        stop=True,
                    )
                    nc.vector.tensor_copy(yt[:, mi, :], py_[:, :256])
                nc.sync.dma_start(
                    y_d[512 * c : 512 * c + 512, :].rearrange(
                        "(t p) c -> p t c", p=128
                    ),
                    yt[:],
                )

    nc.compile()
    return nc


def _get_nc(has_bias: bool):
    if has_bias not in _NC_CACHE:
        _NC_CACHE[has_bias] = _build(has_bias)
    return _NC_CACHE[has_bias]


def _core_inputs(core, x, w_qkv, b_qkv):
    b, g = core // 2, core % 2
    qs, ks, vs = 128 * g, 256 + 128 * g, 512 + 128 * g
    wqk = np.concatenate(
        [w_qkv[:, qs : qs + 128], w_qkv[:, ks : ks + 128]], axis=1
    )
    wva = np.zeros((C, 256), dtype=np.float32)
    vrow = np.zeros((1, 256), dtype=np.float32)
    for pi in range(2):
        for l in range(2):
            h = 2 * pi + l
            col = 128 * pi + 64 * l
            wva[:, col : col + 32] = w_qkv[:, vs + 32 * h : vs + 32 * h + 32]
            vrow[0, col : col + 32] = b_qkv[vs + 32 * h : vs + 32 * h + 32]
            vrow[0, col + 32] = 1.0
    ed = np.zeros((128, 128), dtype=np.float32)
    for h in range(4):
        ed[32 * h, 32 * h : 32 * h + 32] = 1.0
    return {
        "ed": ed,
        "x": np.ascontiguousarray(x[b]).astype(np.float32),
        "wqk": np.ascontiguousarray(wqk).astype(np.float32),
        "wva": wva,
        "vrow": vrow,
        "bq": np.ascontiguousarray(b_qkv[qs : qs + 128]).astype(np.float32)[:, None],
        "bk": np.ascontiguousarray(b_qkv[ks : ks + 128]).astype(np.float32)[:, None],
    }


def kernel(x, w_qkv, b_qkv, w_proj, b_proj):
    from concourse.bass_utils import run_bass_kernel_spmd

    x = np.asarray(x, dtype=np.float32)
    w_qkv = np.asarray(w_qkv, dtype=np.float32)
    b_qkv = np.asarray(b_qkv, dtype=np.float32)
    w_proj = np.asarray(w_proj, dtype=np.float32)
    b_proj = np.asarray(b_proj, dtype=np.float32)
    assert x.shape == (B, T, C), x.shape

    has_bias = bool(np.any(b_qkv))
    nc = _get_nc(has_bias)

    in_maps = []
    for core in range(NCORES):
        m = _core_inputs(core, x, w_qkv, b_qkv)
        g = core % 2
        m["wp"] = np.ascontiguousarray(w_proj[128 * g : 128 * g + 128, :]).astype(
            np.float32
        )
        in_maps.append(m)

    res = run_bass_kernel_spmd(nc, in_maps, list(range(NCORES)))
    y = np.empty((B, T, C), dtype=np.float32)
    for b in range(B):
        y[b] = res.results[2 * b]["y"] + res.results[2 * b + 1]["y"] + b_proj
    return y



# revision 2
# speedup vs baseline: 1.7617x; 1.7617x over previous
"""Causal self-attention (B=4, T=2048, C=256, H=8, HD=32) on 8 NeuronCores.

Sharding: core = 2*b + g (b = batch 0..3, g = head-group 0..1; each group =
4 heads = 128 channels). Each core computes, for its batch b and its 4 heads:
qkv projection (its slice of w_qkv), causal softmax attention, and a partial
output projection y_partial = att_out @ w_proj[group rows, :]. Host sums the
two partials per batch and adds b_proj.

Pipelined design (v1): x is transposed on the host and DMA'd as fp16; all
matmul operands fp16. Main loop issue order per i-tile is
  S(i) -> EXP(i) [ScalarE] -> {AV(i-1), den(i-1)} [PE, hidden under EXP(i)]
so the ScalarE exp stream (the dominant engine load, ~68us) overlaps the
PE attention matmuls. PSUM: S 4 banks (single buffer), AV accum 1 bank
(M=32 dense col-tiled), denominator accum 1 bank (all-ones stationary,
every row of the bank = that head's denominator), 2 spare banks for
qkv/v/proj work which is sprinkled through the i-loop.
Causal mask: DVE multiply by a host-prepared lower-triangular fp16 mask on
the diagonal 128-blocks (post-exp). Normalize: 1-pass reciprocal of the
den bank + one psum*sbuf multiply into otn (fp16).
"""

import math

import numpy as np

B, T, C = 4, 2048, 256
H, HD = 8, 32
NCORES = 8
SCALE = 1.0 / math.sqrt(HD)
NCH = T // 512  # 4 query chunks of 512
NT = T // 128  # 16 tiles of 128

_NC_CACHE = {}


def _build(has_bias: bool):
    import concourse.bass as bass  # noqa: F401
    import concourse.mybir as mybir
    import concourse.tile as tile
    from concourse import bacc

    F32 = mybir.dt.float32
    F16 = mybir.dt.float16
    AF = mybir.ActivationFunctionType
    ALU = mybir.AluOpType

    nc = bacc.Bacc("TRN2", target_bir_lowering=False, debug=False, num_devices=NCORES)

    xT_d = nc.declare_dram_parameter("xT", [C, T], F16, isOutput=False)
    wqk_d = nc.declare_dram_parameter("wqk", [C, 256], F16, isOutput=False)
    wva_d = nc.declare_dram_parameter("wva", [C, 128], F16, isOutput=False)
    wp_d = nc.declare_dram_parameter("wp", [128, C], F16, isOutput=False)
    mk_d = nc.declare_dram_parameter("mk", [128, 512], F16, isOutput=False)
    bq_d = nc.declare_dram_parameter("bq", [128, 1], F32, isOutput=False)
    bk_d = nc.declare_dram_parameter("bk", [128, 1], F32, isOutput=False)
    bv_d = nc.declare_dram_parameter("bv", [1, 128], F32, isOutput=False)
    y_d = nc.declare_dram_parameter("y", [T, C], F16, isOutput=True)

    with tile.TileContext(nc) as tc:
        with (
            tc.tile_pool(name="cst", bufs=1) as cst,
            tc.tile_pool(name="expp", bufs=3) as expp,
            tc.tile_pool(name="smt", bufs=2) as smt,
            tc.tile_pool(name="yout", bufs=2) as yout,
            tc.tile_pool(name="psS", bufs=1, space="PSUM") as psS,
            tc.tile_pool(name="psv", bufs=1, space="PSUM") as psv,
            tc.tile_pool(name="psd", bufs=1, space="PSUM") as psd,
            tc.tile_pool(name="pss", bufs=2, space="PSUM") as pss,
        ):
            # ---------------- setup ----------------
            # force the exp table load to overlap the input DMAs
            dums = cst.tile([128, 1], F32)
            dumm = cst.tile([128, 1], F16)
            nc.vector.memset(dums[:], 0.0)
            nc.scalar.activation(dumm[:], dums[:], AF.Exp, scale=1.0)

            ones32 = cst.tile([128, 128], F16)
            nc.vector.memset(ones32[:], 1.0)
            mask3 = cst.tile([128, 4, 128], F16)
            nc.sync.dma_start(mask3[:], mk_d[:].rearrange("p (h q) -> p h q", h=4))

            wqk_sb = cst.tile([128, 2, 256], F16)
            wva_sb = cst.tile([128, 2, 128], F16)
            wp_sb = cst.tile([128, 256], F16)
            nc.sync.dma_start(wqk_sb[:], wqk_d[:].rearrange("(a p) m -> p a m", p=128))
            nc.sync.dma_start(wva_sb[:], wva_d[:].rearrange("(a p) m -> p a m", p=128))
            nc.sync.dma_start(wp_sb[:], wp_d[:])

            if has_bias:
                bq_sb = cst.tile([128, 1], F32)
                bk_sb = cst.tile([128, 1], F32)
                nc.sync.dma_start(bq_sb[:], bq_d[:])
                nc.sync.dma_start(bk_sb[:], bk_d[:])
                bv_sb = cst.tile([1, 128], F32)
                nc.sync.dma_start(bv_sb[:], bv_d[:])
                ones1 = cst.tile([1, 128], F32)
                nc.vector.memset(ones1[:], 1.0)
                pbv = pss.tile([128, 512], F32, tag="sm")
                nc.tensor.matmul(pbv[:, :128], ones1[:], bv_sb[:], start=True, stop=True)
                bvb = cst.tile([128, 128], F32)
                nc.vector.tensor_copy(bvb[:], pbv[:, :128])
            else:
                bq_sb = bk_sb = bvb = None

            # x^T arrives pre-transposed from the host, in 4 column slices so
            # chunk 0 compute can start as soon as its slice lands
            xT = cst.tile([128, 2, T], F16)
            for cc in range(NCH):
                nc.sync.dma_start(
                    xT[:, :, 512 * cc : 512 * cc + 512],
                    xT_d[:, 512 * cc : 512 * cc + 512].rearrange(
                        "(a p) t -> p a t", p=128
                    ),
                )

            qT = cst.tile([128, T], F16)
            kT = cst.tile([128, T], F16)
            vaug = cst.tile([128, NT, 128], F16)
            otn = cst.tile([128, T], F16)

            def qk_chunk(n):
                for dest, bias, wc in ((qT, bq_sb, 0), (kT, bk_sb, 128)):
                    pq = pss.tile([128, 512], F32, tag="sm")
                    for ko in range(2):
                        nc.tensor.matmul(
                            pq[:],
                            wqk_sb[:, ko, wc : wc + 128],
                            xT[:, ko, 512 * n : 512 * n + 512],
                            start=(ko == 0),
                            stop=(ko == 1),
                        )
                    if has_bias:
                        nc.vector.tensor_scalar(
                            dest[:, 512 * n : 512 * n + 512],
                            pq[:],
                            bias[:, 0:1],
                            None,
                            ALU.add,
                        )
                    else:
                        nc.vector.tensor_copy(dest[:, 512 * n : 512 * n + 512], pq[:])

            def v_tile(t):
                pv_ = pss.tile([128, 512], F32, tag="sm")
                for ko in range(2):
                    nc.tensor.matmul(
                        pv_[:, :128],
                        xT[:, ko, 128 * t : 128 * t + 128],
                        wva_sb[:, ko, :],
                        start=(ko == 0),
                        stop=(ko == 1),
                    )
                if has_bias:
                    nc.vector.tensor_tensor(vaug[:, t, :], pv_[:, :128], bvb[:], ALU.add)
                else:
                    nc.vector.tensor_copy(vaug[:, t, :], pv_[:, :128])

            # ---------------- main attention loop ----------------
            qk_chunk(0)
            for t in range(4):
                v_tile(t)

            for c in range(NCH):
                ilast = 4 * c + 3
                pv = psv.tile([128, 512], F32, tag="pv")
                pd = psd.tile([128, 512], F32, tag="pd")

                def av_den(prev):
                    i, ex, off = prev
                    for h in range(4):
                        nc.tensor.matmul(
                            pv[32 * h : 32 * h + 32, off:512],
                            vaug[:, i, 32 * h : 32 * h + 32],
                            ex[:, h, off:512],
                            start=(i == 0),
                            stop=(i == ilast),
                            tile_position=(0, 32 * h),
                        )
                    for h in range(4):
                        nc.tensor.matmul(
                            pd[32 * h : 32 * h + 32, off:512],
                            ones32[:, 32 * h : 32 * h + 32],
                            ex[:, h, off:512],
                            start=(i == 0),
                            stop=(i == ilast),
                            tile_position=(0, 32 * h),
                        )

                # next chunk's q/k/v work, sprinkled one piece per iteration
                prol = []
                if c + 1 < NCH:
                    prol = [lambda n=c + 1: qk_chunk(n)] + [
                        lambda t=t: v_tile(t)
                        for t in range(4 * (c + 1), 4 * (c + 1) + 4)
                    ]

                prev = None
                for i in range(0, ilast + 1):
                    off = max(0, (i - 4 * c) * 128)
                    sp = psS.tile([128, 4, 512], F32, tag="S")
                    for h in range(4):
                        nc.tensor.matmul(
                            sp[:, h, off:512],
                            kT[32 * h : 32 * h + 32, 128 * i : 128 * i + 128],
                            qT[
                                32 * h : 32 * h + 32,
                                512 * c + off : 512 * c + 512,
                            ],
                            start=True,
                            stop=True,
                            tile_position=(32 * h, 0),
                        )
                    ex = expp.tile([128, 4, 512], F16, tag="ex")
                    nc.scalar.activation(
                        ex[:, :, off:512], sp[:, :, off:512], AF.Exp, scale=SCALE
                    )
                    if i >= 4 * c:
                        nc.vector.tensor_tensor(
                            ex[:, :, off : off + 128],
                            ex[:, :, off : off + 128],
                            mask3[:],
                            ALU.mult,
                        )
                    if prev is not None:
                        av_den(prev)
                    if prol:
                        prol.pop(0)()
                    prev = (i, ex, off)
                av_den(prev)
                while prol:
                    prol.pop(0)()

                # ---- epilogue: normalize + project this chunk ----
                dr = smt.tile([128, 512], F32, tag="dr")
                nc.vector.reciprocal_approx_fast(dr[:], pd[:])
                nc.vector.tensor_tensor(
                    otn[:, 512 * c : 512 * c + 512], pv[:], dr[:], ALU.mult
                )
                yt = yout.tile([128, 4, 256], F16, tag="yt")
                for mi in range(4):
                    m = 4 * c + mi
                    py = pss.tile([128, 512], F32, tag="sm")
                    nc.tensor.matmul(
                        py[:, :256],
                        otn[:, 128 * m : 128 * m + 128],
                        wp_sb[:],
                        start=True,
                        stop=True,
                    )
                    nc.vector.tensor_copy(yt[:, mi, :], py[:, :256])
                nc.sync.dma_start(
                    y_d[512 * c : 512 * c + 512, :].rearrange(
                        "(t p) c -> p t c", p=128
                    ),
                    yt[:],
                )

    nc.compile()
    return nc


def _get_nc(has_bias: bool):
    if has_bias not in _NC_CACHE:
        _NC_CACHE[has_bias] = _build(has_bias)
    return _NC_CACHE[has_bias]


_MASK = None


def _tri_mask():
    global _MASK
    if _MASK is None:
        m = np.triu(np.ones((128, 128), dtype=np.float16))  # keep key<=query
        _MASK = np.ascontiguousarray(np.tile(m, (1, 4)))  # [128, 512]
    return _MASK


def _core_inputs(core, x, w_qkv, b_qkv, w_proj):
    b, g = core // 2, core % 2
    qs, ks, vs = 128 * g, 256 + 128 * g, 512 + 128 * g
    wqk = np.concatenate(
        [w_qkv[:, qs : qs + 128], w_qkv[:, ks : ks + 128]], axis=1
    )
    return {
        "xT": np.ascontiguousarray(x[b].T).astype(np.float16),
        "wqk": np.ascontiguousarray(wqk).astype(np.float16),
        "wva": np.ascontiguousarray(w_qkv[:, vs : vs + 128]).astype(np.float16),
        "wp": np.ascontiguousarray(w_proj[128 * g : 128 * g + 128, :]).astype(
            np.float16
        ),
        "mk": _tri_mask(),
        "bq": np.ascontiguousarray(b_qkv[qs : qs + 128]).astype(np.float32)[:, None],
        "bk": np.ascontiguousarray(b_qkv[ks : ks + 128]).astype(np.float32)[:, None],
        "bv": np.ascontiguousarray(b_qkv[vs : vs + 128]).astype(np.float32)[None, :],
    }


def _in_maps(x, w_qkv, b_qkv, w_proj):
    return [_core_inputs(core, x, w_qkv, b_qkv, w_proj) for core in range(NCORES)]


def kernel(x, w_qkv, b_qkv, w_proj, b_proj):
    from concourse.bass_utils import run_bass_kernel_spmd

    x = np.asarray(x, dtype=np.float32)
    w_qkv = np.asarray(w_qkv, dtype=np.float32)
    b_qkv = np.asarray(b_qkv, dtype=np.float32)
    w_proj = np.asarray(w_proj, dtype=np.float32)
    b_proj = np.asarray(b_proj, dtype=np.float32)
    assert x.shape == (B, T, C), x.shape

    has_bias = bool(np.any(b_qkv))
    nc = _get_nc(has_bias)

    res = run_bass_kernel_spmd(
        nc, _in_maps(x, w_qkv, b_qkv, w_proj), list(range(NCORES))
    )
    y = np.empty((B, T, C), dtype=np.float32)
    for b in range(B):
        y[b] = (
            res.results[2 * b]["y"].astype(np.float32)
            + res.results[2 * b + 1]["y"].astype(np.float32)
            + b_proj
        )
    return y


# revision 3
# speedup vs baseline: 1.9501x; 1.1070x over previous
"""Causal self-attention (B=4, T=2048, C=256, H=8, HD=32) on 8 NeuronCores.

Sharding: core = 2*b + g (b = batch 0..3, g = head-group 0..1; each group =
4 heads = 128 channels). Each core: qkv projection (its slice of w_qkv),
causal softmax attention for its 4 heads, partial output projection with
w_proj[group rows, :]. Host sums the two partials per batch and adds b_proj.

v2: ScalarE-saturated pipeline. S and EXP are split into 2-head halves so
the exp stream never waits for PE: S_h01(i+1) writes its 2 PSUM banks while
EXP_b(i) reads the other half's banks, and vice versa. AV/denominator
matmuls are issued as two 4-strip col-tiled waves (AV_h01+den_h23, then
AV_h23+den_h01) hidden under the EXP windows, accumulating into 1 bank each
(M=32 dense AV; all-ones stationary for denominators so every row of the
den bank is that head's softmax denominator). Causal mask = DVE multiply by
a host fp16 triangle on diagonal blocks. Normalize = 1-pass reciprocal +
one psum*sbuf multiply. All matmul operands fp16; x arrives pre-transposed
from the host; DMA layouts are packed so descriptors are 2KB/partition.
"""

import math

import numpy as np

B, T, C = 4, 2048, 256
H, HD = 8, 32
NCORES = 8
SCALE = 1.0 / math.sqrt(HD)
NCH = T // 512  # 4 query chunks of 512
NT = T // 128  # 16 tiles of 128

_NC_CACHE = {}


def _build(has_bias: bool):
    import concourse.bass as bass  # noqa: F401
    import concourse.mybir as mybir
    import concourse.tile as tile
    from concourse import bacc

    F32 = mybir.dt.float32
    F16 = mybir.dt.float16
    AF = mybir.ActivationFunctionType
    ALU = mybir.AluOpType

    nc = bacc.Bacc("TRN2", target_bir_lowering=False, debug=False, num_devices=NCORES)

    xT_d = nc.declare_dram_parameter("xT", [128, 2 * T], F16, isOutput=False)
    wqk_d = nc.declare_dram_parameter("wqk", [128, 512], F16, isOutput=False)
    wva_d = nc.declare_dram_parameter("wva", [128, 256], F16, isOutput=False)
    wp_d = nc.declare_dram_parameter("wp", [128, C], F16, isOutput=False)
    mk_d = nc.declare_dram_parameter("mk", [128, 512], F16, isOutput=False)
    bq_d = nc.declare_dram_parameter("bq", [128, 1], F32, isOutput=False)
    bk_d = nc.declare_dram_parameter("bk", [128, 1], F32, isOutput=False)
    bv_d = nc.declare_dram_parameter("bv", [1, 128], F32, isOutput=False)
    y_d = nc.declare_dram_parameter("y", [128, 2 * T], F16, isOutput=True)

    with tile.TileContext(nc) as tc:
        with (
            tc.tile_pool(name="cst", bufs=1) as cst,
            tc.tile_pool(name="expp", bufs=3) as expp,
            tc.tile_pool(name="smt", bufs=2) as smt,
            tc.tile_pool(name="yout", bufs=2) as yout,
            tc.tile_pool(name="psA", bufs=1, space="PSUM") as psA,
            tc.tile_pool(name="psB", bufs=1, space="PSUM") as psB,
            tc.tile_pool(name="psv", bufs=1, space="PSUM") as psv,
            tc.tile_pool(name="psd", bufs=1, space="PSUM") as psd,
            tc.tile_pool(name="pss", bufs=2, space="PSUM") as pss,
        ):
            # ---------------- setup ----------------
            # force the exp table load to overlap the input DMAs
            dums = cst.tile([128, 1], F32)
            dumm = cst.tile([128, 1], F16)
            nc.vector.memset(dums[:], 0.0)
            nc.scalar.activation(dumm[:], dums[:], AF.Exp, scale=1.0)

            # x^T first: chunk-0 compute is gated on slice 0 + wqk
            xT = cst.tile([128, 2, T], F16)
            nc.sync.dma_start(
                xT[:, :, 0:512], xT_d[:, 0:1024].rearrange("p (a t) -> p a t", a=2)
            )
            wqk_sb = cst.tile([128, 2, 256], F16)
            nc.sync.dma_start(wqk_sb[:], wqk_d[:].rearrange("p (a m) -> p a m", a=2))
            wva_sb = cst.tile([128, 2, 128], F16)
            nc.sync.dma_start(wva_sb[:], wva_d[:].rearrange("p (a m) -> p a m", a=2))
            for cc in range(1, NCH):
                nc.sync.dma_start(
                    xT[:, :, 512 * cc : 512 * cc + 512],
                    xT_d[:, 1024 * cc : 1024 * cc + 1024].rearrange(
                        "p (a t) -> p a t", a=2
                    ),
                )
            wp_sb = cst.tile([128, 256], F16)
            nc.sync.dma_start(wp_sb[:], wp_d[:])
            mask3 = cst.tile([128, 4, 128], F16)
            nc.sync.dma_start(mask3[:], mk_d[:].rearrange("p (h q) -> p h q", h=4))
            ones32 = cst.tile([128, 128], F16)
            nc.vector.memset(ones32[:], 1.0)

            if has_bias:
                bq_sb = cst.tile([128, 1], F32)
                bk_sb = cst.tile([128, 1], F32)
                nc.sync.dma_start(bq_sb[:], bq_d[:])
                nc.sync.dma_start(bk_sb[:], bk_d[:])
                bv_sb = cst.tile([1, 128], F32)
                nc.sync.dma_start(bv_sb[:], bv_d[:])
                ones1 = cst.tile([1, 128], F32)
                nc.vector.memset(ones1[:], 1.0)
                pbv = pss.tile([128, 512], F32, tag="sm")
                nc.tensor.matmul(pbv[:, :128], ones1[:], bv_sb[:], start=True, stop=True)
                bvb = cst.tile([128, 128], F32)
                nc.vector.tensor_copy(bvb[:], pbv[:, :128])
            else:
                bq_sb = bk_sb = bvb = None

            qT = cst.tile([128, T], F16)
            kT = cst.tile([128, T], F16)
            vaug = cst.tile([128, NT, 128], F16)
            otn = cst.tile([128, T], F16)

            def qk_chunk(n):
                for dest, bias, wc in ((qT, bq_sb, 0), (kT, bk_sb, 128)):
                    pq = pss.tile([128, 512], F32, tag="sm")
                    for ko in range(2):
                        nc.tensor.matmul(
                            pq[:],
                            wqk_sb[:, ko, wc : wc + 128],
                            xT[:, ko, 512 * n : 512 * n + 512],
                            start=(ko == 0),
                            stop=(ko == 1),
                        )
                    if has_bias:
                        nc.vector.tensor_scalar(
                            dest[:, 512 * n : 512 * n + 512],
                            pq[:],
                            bias[:, 0:1],
                            None,
                            ALU.add,
                        )
                    else:
                        nc.vector.tensor_copy(dest[:, 512 * n : 512 * n + 512], pq[:])

            def v_tile(t):
                pv_ = pss.tile([128, 512], F32, tag="sm")
                for ko in range(2):
                    nc.tensor.matmul(
                        pv_[:, :128],
                        xT[:, ko, 128 * t : 128 * t + 128],
                        wva_sb[:, ko, :],
                        start=(ko == 0),
                        stop=(ko == 1),
                    )
                if has_bias:
                    nc.vector.tensor_tensor(vaug[:, t, :], pv_[:, :128], bvb[:], ALU.add)
                else:
                    nc.vector.tensor_copy(vaug[:, t, :], pv_[:, :128])

            # ---------------- main attention loop ----------------
            qk_chunk(0)
            for t in range(4):
                v_tile(t)

            for c in range(NCH):
                ilast = 4 * c + 3
                pv = psv.tile([128, 512], F32, tag="pv")
                pd = psd.tile([128, 512], F32, tag="pd")

                def one_av(kind, h, prev):
                    i, ex, off = prev
                    dst, lhs = (
                        (pv, vaug[:, i, 32 * h : 32 * h + 32])
                        if kind == "av"
                        else (pd, ones32[:, 32 * h : 32 * h + 32])
                    )
                    nc.tensor.matmul(
                        dst[32 * h : 32 * h + 32, off:512],
                        lhs,
                        ex[:, h, off:512],
                        start=(i == 0),
                        stop=(i == ilast),
                        tile_position=(0, 32 * h),
                    )

                def wave1(prev):
                    # strips 0,1 AV (h0,h1) + strips 2,3 den (h2,h3): 4
                    # distinct PE column groups -> fully concurrent
                    one_av("av", 0, prev)
                    one_av("av", 1, prev)
                    one_av("den", 2, prev)
                    one_av("den", 3, prev)

                def wave2(prev):
                    one_av("den", 0, prev)
                    one_av("den", 1, prev)
                    one_av("av", 2, prev)
                    one_av("av", 3, prev)

                def s_half(i, off, sp, hs):
                    for h in hs:
                        nc.tensor.matmul(
                            sp[:, h % 2, off:512],
                            kT[32 * h : 32 * h + 32, 128 * i : 128 * i + 128],
                            qT[
                                32 * h : 32 * h + 32,
                                512 * c + off : 512 * c + 512,
                            ],
                            start=True,
                            stop=True,
                            tile_position=(32 * h, 0),
                        )

                # next chunk's q/k/v work, sprinkled one piece per iteration
                prol = []
                if c + 1 < NCH:
                    prol = [lambda n=c + 1: qk_chunk(n)] + [
                        lambda t=t: v_tile(t)
                        for t in range(4 * (c + 1), 4 * (c + 1) + 4)
                    ]

                prev = None
                for i in range(0, ilast + 1):
                    off = max(0, (i - 4 * c) * 128)
                    spA = psA.tile([128, 2, 512], F32, tag="SA")
                    s_half(i, off, spA, (0, 1))
                    ex = expp.tile([128, 4, 512], F16, tag="ex")
                    nc.scalar.activation(
                        ex[:, 0:2, off:512], spA[:, :, off:512], AF.Exp, scale=SCALE
                    )
                    if i >= 4 * c:
                        nc.vector.tensor_tensor(
                            ex[:, 0:2, off : off + 128],
                            ex[:, 0:2, off : off + 128],
                            mask3[:, 0:2, :],
                            ALU.mult,
                        )
                    if prev is not None:
                        wave1(prev)
                    spB = psB.tile([128, 2, 512], F32, tag="SB")
                    s_half(i, off, spB, (2, 3))
                    nc.scalar.activation(
                        ex[:, 2:4, off:512], spB[:, :, off:512], AF.Exp, scale=SCALE
                    )
                    if i >= 4 * c:
                        nc.vector.tensor_tensor(
                            ex[:, 2:4, off : off + 128],
                            ex[:, 2:4, off : off + 128],
                            mask3[:, 2:4, :],
                            ALU.mult,
                        )
                    if prev is not None:
                        wave2(prev)
                        if prol:
                            prol.pop(0)()
                    prev = (i, ex, off)
                wave1(prev)
                wave2(prev)
                while prol:
                    prol.pop(0)()

                # ---- epilogue: normalize + project this chunk ----
                dr = smt.tile([128, 512], F32, tag="dr")
                nc.vector.reciprocal_approx_fast(dr[:], pd[:])
                nc.vector.tensor_tensor(
                    otn[:, 512 * c : 512 * c + 512], pv[:], dr[:], ALU.mult
                )
                yt = yout.tile([128, 4, 256], F16, tag="yt")
                for half in range(2):
                    for mi in range(2 * half, 2 * half + 2):
                        m = 4 * c + mi
                        py = pss.tile([128, 512], F32, tag="sm")
                        nc.tensor.matmul(
                            py[:, :256],
                            otn[:, 128 * m : 128 * m + 128],
                            wp_sb[:],
                            start=True,
                            stop=True,
                        )
                        nc.vector.tensor_copy(yt[:, mi, :], py[:, :256])
                    nc.sync.dma_start(
                        y_d[:, 1024 * c + 512 * half : 1024 * c + 512 * half + 512],
                        yt[:, 2 * half : 2 * half + 2, :],
                    )

    nc.compile()
    return nc


def _get_nc(has_bias: bool):
    if has_bias not in _NC_CACHE:
        _NC_CACHE[has_bias] = _build(has_bias)
    return _NC_CACHE[has_bias]


_MASK = None


def _tri_mask():
    global _MASK
    if _MASK is None:
        m = np.triu(np.ones((128, 128), dtype=np.float16))  # keep key<=query
        _MASK = np.ascontiguousarray(np.tile(m, (1, 4)))  # [128, 512]
    return _MASK


def _core_inputs(core, x, w_qkv, b_qkv, w_proj):
    b, g = core // 2, core % 2
    qs, ks, vs = 128 * g, 256 + 128 * g, 512 + 128 * g
    # x^T packed as [128, c, a, 512]: row p, col block (c,a) = x[b].T[128a+p, 512c:]
    xt = np.ascontiguousarray(x[b].T).astype(np.float16)  # [256, 2048]
    xh = xt.reshape(2, 128, NCH, 512).transpose(1, 2, 0, 3).reshape(128, 2 * T)
    wqk = np.concatenate(
        [w_qkv[:, qs : qs + 128], w_qkv[:, ks : ks + 128]], axis=1
    ).astype(np.float16)  # [256, 256]
    wqkh = wqk.reshape(2, 128, 256).transpose(1, 0, 2).reshape(128, 512)
    wva = np.ascontiguousarray(w_qkv[:, vs : vs + 128]).astype(np.float16)
    wvah = wva.reshape(2, 128, 128).transpose(1, 0, 2).reshape(128, 256)
    return {
        "xT": np.ascontiguousarray(xh),
        "wqk": np.ascontiguousarray(wqkh),
        "wva": np.ascontiguousarray(wvah),
        "wp": np.ascontiguousarray(w_proj[128 * g : 128 * g + 128, :]).astype(
            np.float16
        ),
        "mk": _tri_mask(),
        "bq": np.ascontiguousarray(b_qkv[qs : qs + 128]).astype(np.float32)[:, None],
        "bk": np.ascontiguousarray(b_qkv[ks : ks + 128]).astype(np.float32)[:, None],
        "bv": np.ascontiguousarray(b_qkv[vs : vs + 128]).astype(np.float32)[None, :],
    }


def _in_maps(x, w_qkv, b_qkv, w_proj):
    return [_core_inputs(core, x, w_qkv, b_qkv, w_proj) for core in range(NCORES)]


def kernel(x, w_qkv, b_qkv, w_proj, b_proj):
    from concourse.bass_utils import run_bass_kernel_spmd

    x = np.asarray(x, dtype=np.float32)
    w_qkv = np.asarray(w_qkv, dtype=np.float32)
    b_qkv = np.asarray(b_qkv, dtype=np.float32)
    w_proj = np.asarray(w_proj, dtype=np.float32)
    b_proj = np.asarray(b_proj, dtype=np.float32)
    assert x.shape == (B, T, C), x.shape

    has_bias = bool(np.any(b_qkv))
    nc = _get_nc(has_bias)

    res = run_bass_kernel_spmd(
        nc, _in_maps(x, w_qkv, b_qkv, w_proj), list(range(NCORES))
    )
    y = np.empty((B, T, C), dtype=np.float32)
    for b in range(B):
        # y dram layout: [128, c, t, 256] -> rows 512c+128t+p
        acc = None
        for part in (res.results[2 * b]["y"], res.results[2 * b + 1]["y"]):
            yb = (
                part.astype(np.float32)
                .reshape(128, NCH, 4, 256)
                .transpose(1, 2, 0, 3)
                .reshape(T, C)
            )
            acc = yb if acc is None else acc + yb
        y[b] = acc + b_proj
    return y
